# revision 19
# baseline (speedup 1.0000x reference)
"""Trainium2 Bass kernel for y = inputs @ weights.T + bias.

Shapes: inputs [8192, 4096] f32, weights [4096, 4096] f32, bias [4096] f32,
output [8192, 4096] f32.

Strategy:
- Data-parallel across 8 NeuronCores: each core computes 1024 rows of the
  output; weights/bias are replicated.
- Host pre-transposes inputs and weights to K-major layout and converts them
  to bf16 (rel err ~2e-3 at K=4096, tolerance is 2e-2). bf16 halves DMA
  traffic/SBUF footprint vs fp32r and enables Fast Weight Load on the PE
  (LDWEIGHTS ~2x faster), while matmul throughput is the same 1 col/cycle.
- Per core: cache the x-slice KxM [4096,1024] bf16 in SBUF (8.4 MB), stream
  W in [128,512] bf16 tiles, 8 PSUM banks accumulate fp32 over K, bias added
  on DVE during PSUM eviction, y written back as fp32.
- DMA queues are dedicated per stream (w: sync+scalar, x: gpsimd+vector,
  y: gpsimd) so the w-tile stream never suffers head-of-line blocking
  behind x/y transfers (which caused a periodic one-matmul stall).
- A short burst of dummy matmuls on a memset tile right after the engine
  preamble warms the PE HAM clock gate (K=4/8 -> 8/8) while the first real
  tiles are still in flight, so real matmuls run at 2.4 GHz from the start.
- The last n-block runs mb-major on a prefetched w-set so 7 of its 8 PSUM
  evictions/output DMAs overlap the matmul stream; only the final [128,512]
  tile drains after the last matmul (split in halves to pipeline with DMA).
"""

import numpy as np
import ml_dtypes

import concourse.bacc as bacc
import concourse.mybir as mybir
import concourse.tile as tile
from concourse.bass_utils import run_bass_kernel_spmd

N_CORES = 8
N_FULL = 8192  # input rows
K_DIM = 4096  # contraction (in features)
O_DIM = 4096  # out features
M = N_FULL // N_CORES  # rows per core (1024)
P = 128
KO = K_DIM // P  # 32 k-tiles
N_TILE = 512  # moving free dim per matmul (1 PSUM bank of fp32)
N_BLOCKS = O_DIM // N_TILE  # 8
M_BLOCKS = M // P  # 8
N_WARM = 17  # dummy warm-up matmuls (~3.7us at the cold 1.2 GHz clock)

_nc_cache = None


def _build():
    nc = bacc.Bacc(target_bir_lowering=False)

    xT = nc.dram_tensor("xT", [K_DIM, M], mybir.dt.bfloat16, kind="ExternalInput")
    wT = nc.dram_tensor("wT", [K_DIM, O_DIM], mybir.dt.bfloat16, kind="ExternalInput")
    biasr = nc.dram_tensor("biasr", [P, O_DIM], mybir.dt.float32, kind="ExternalInput")
    y = nc.dram_tensor("y", [M, O_DIM], mybir.dt.float32, kind="ExternalOutput")

    xT3 = xT.ap().rearrange("(ko p) m -> p ko m", p=P)
    wT3 = wT.ap().rearrange("(ko p) n -> p ko n", p=P)
    y3 = y.ap().rearrange("(mb p) n -> p mb n", p=P)

    with tile.TileContext(nc) as tc:
        with (
            tc.tile_pool(name="persist", bufs=1) as persist,
            tc.tile_pool(name="wpool", bufs=10) as wpool,
            tc.tile_pool(name="opool", bufs=8) as opool,
            tc.tile_pool(name="ohpool", bufs=4) as ohpool,
            tc.tile_pool(name="psum", bufs=1, space="PSUM") as psum_pool,
        ):
            psums = [
                psum_pool.tile(
                    [P, N_TILE], mybir.dt.float32, tag=f"ps{m}", name=f"ps{m}"
                )
                for m in range(M_BLOCKS)
            ]

            # PE warm-up: dummy matmuls on a memset tile, no DMA deps. They
            # run during the DMA latency of the first real tiles and flip the
            # HAM clock gate to full speed before real matmuls start.
            warm = persist.tile([P, 384], mybir.dt.bfloat16, tag="warm")
            nc.gpsimd.memset(warm[:], 0)
            for _ in range(N_WARM):
                nc.tensor.matmul(
                    psums[M_BLOCKS - 1][:, :256],
                    warm[:, :P],
                    warm[:, P:],
                    start=True,
                    stop=True,
                )

            # x cached in SBUF, one tile per k-slab so matmuls can start as
            # soon as their slab has landed. Only gpsimd/sync/scalar can
            # issue DMAs; sync+scalar are dedicated to the w stream so it
            # never queues behind x/y transfers. x rides gpsimd alone,
            # except slab 0's odd chunks which go out on scalar before any
            # w traffic exists there, halving the first matmul's wait.
            # The first four slabs arrive in fine-grained chunks split over
            # gpsimd+scalar (scalar's first w tile was moved to sync, so its
            # early window is free) - a late early slab would idle the PE
            # past the HAM window and trigger an expensive re-throttle.
            x_sb = []
            bias_sb = [None] * N_BLOCKS
            x_chunks = {0: 4, 1: 2, 2: 2, 3: 2, 4: 2, 5: 2}  # ko -> chunks
            for ko in range(KO):
                x_t = persist.tile([P, M], mybir.dt.bfloat16, tag=f"x{ko}")
                nchunk = x_chunks.get(ko, 1)
                csz = M // nchunk
                for c in range(nchunk):
                    xeng = nc.scalar if c % 2 == 1 else nc.gpsimd
                    xeng.dma_start(
                        x_t[:, c * csz : (c + 1) * csz],
                        xT3[:, ko, c * csz : (c + 1) * csz],
                    )
                x_sb.append(x_t)

            # bias, then the whole w-set of the last n-block, prefetched on
            # gpsimd (idle once x is cached) so nb=7 can run mb-major with
            # zero DMA dependencies.
            for nb in range(N_BLOCKS):
                b_t = persist.tile([P, N_TILE], mybir.dt.float32, tag=f"bias{nb}")
                nc.gpsimd.dma_start(
                    b_t[:], biasr.ap()[:, nb * N_TILE : (nb + 1) * N_TILE]
                )
                bias_sb[nb] = b_t

            LAST = N_BLOCKS - 1
            w_last = []

            # n-blocks 0..6: ko-major, streaming w tiles on sync/scalar.
            for nb in range(N_BLOCKS - 1):
                if nb == 1:
                    # prefetch the last n-block's w set now: the x load is
                    # done and gpsimd is idle, whereas during nb=0 the
                    # per-core HBM demand is already at its cap.
                    for ko in range(KO):
                        wl_t = persist.tile(
                            [P, N_TILE], mybir.dt.bfloat16, tag=f"wl{ko}"
                        )
                        nc.gpsimd.dma_start(
                            wl_t[:], wT3[:, ko, LAST * N_TILE : (LAST + 1) * N_TILE]
                        )
                        w_last.append(wl_t)
                for ko in range(KO):
                    w_t = wpool.tile([P, N_TILE], mybir.dt.bfloat16, tag="w")
                    # ko parity splits w over sync/scalar; w01/w03 go to sync
                    # too so scalar's early x chunks don't delay them.
                    weng = nc.sync if (ko % 2 == 0 or (nb == 0 and ko in (1, 3))) else nc.scalar
                    if nb == 0 and ko == 0:
                        # halve the very first tile's DMA latency
                        h = N_TILE // 2
                        weng.dma_start(w_t[:, :h], wT3[:, ko, :h])
                        weng.dma_start(w_t[:, h:], wT3[:, ko, h : N_TILE])
                    else:
                        weng.dma_start(
                            w_t[:], wT3[:, ko, nb * N_TILE : (nb + 1) * N_TILE]
                        )
                    for mb in range(M_BLOCKS):
                        nc.tensor.matmul(
                            psums[mb][:],
                            x_sb[ko][:, mb * P : (mb + 1) * P],
                            w_t[:],
                            start=(ko == 0),
                            stop=(ko == KO - 1),
                        )
                for mb in range(M_BLOCKS):
                    o_t = opool.tile([P, N_TILE], mybir.dt.float32, tag="o")
                    nc.vector.tensor_add(o_t[:], psums[mb][:], bias_sb[nb][:])
                    nc.gpsimd.dma_start(
                        y3[:, mb, nb * N_TILE : (nb + 1) * N_TILE], o_t[:]
                    )

            # Last n-block: mb-major over the prefetched w-set. Each psum
            # bank finishes 32 matmuls before the next starts, so its
            # eviction + y DMA overlap the remaining matmuls; only mb=7
            # drains after the final matmul, in halves to pipeline with DMA.
            H = N_TILE // 2
            for mb in range(M_BLOCKS):
                for ko in range(KO):
                    if mb == M_BLOCKS - 1 and ko == KO - 1:
                        # split the final matmul in halves so the first
                        # half's eviction overlaps the second half
                        for h in range(2):
                            nc.tensor.matmul(
                                psums[mb][:, h * H : (h + 1) * H],
                                x_sb[ko][:, mb * P : (mb + 1) * P],
                                w_last[ko][:, h * H : (h + 1) * H],
                                start=False,
                                stop=True,
                            )
                        continue
                    nc.tensor.matmul(
                        psums[mb][:],
                        x_sb[ko][:, mb * P : (mb + 1) * P],
                        w_last[ko][:],
                        start=(ko == 0),
                        stop=(ko == KO - 1),
                    )
                for h in range(2):
                    o_t = ohpool.tile([P, H], mybir.dt.float32, tag="oh")
                    nc.vector.tensor_add(
                        o_t[:],
                        psums[mb][:, h * H : (h + 1) * H],
                        bias_sb[LAST][:, h * H : (h + 1) * H],
                    )
                    oeng = nc.sync if h % 2 == 0 else nc.scalar
                    oeng.dma_start(
                        y3[
                            :,
                            mb,
                            LAST * N_TILE + h * H : LAST * N_TILE + (h + 1) * H,
                        ],
                        o_t[:],
                    )

    nc.compile()
    return nc


def _get_nc():
    global _nc_cache
    if _nc_cache is None:
        _nc_cache = _build()
    return _nc_cache


def _make_in_maps(inputs, weights, bias):
    x = np.asarray(inputs, dtype=np.float32)
    w = np.asarray(weights, dtype=np.float32)
    b = np.asarray(bias, dtype=np.float32)

    xbf = np.ascontiguousarray(x).astype(ml_dtypes.bfloat16)
    wbf = np.ascontiguousarray(w).astype(ml_dtypes.bfloat16)
    xT = xbf.T  # [K, N_FULL] view
    wT = np.ascontiguousarray(wbf.T)  # [K, O]
    br = np.ascontiguousarray(np.broadcast_to(b[None, :], (P, O_DIM)))

    in_maps = []
    for c in range(N_CORES):
        xTc = np.ascontiguousarray(xT[:, c * M : (c + 1) * M])
        in_maps.append({"xT": xTc, "wT": wT, "biasr": br})
    return in_maps


def kernel(**inputs):
    nc = _get_nc()
    in_maps = _make_in_maps(inputs["inputs"], inputs["weights"], inputs["bias"])
    res = run_bass_kernel_spmd(nc, in_maps, core_ids=list(range(N_CORES)))
    return np.concatenate([r["y"] for r in res.results], axis=0)


def run_traced(inputs, weights, bias, **trace_kwargs):
    """Used by test.py: same computation, returns (output, BassKernelResults)."""
    nc = _get_nc()
    in_maps = _make_in_maps(inputs, weights, bias)
    res = run_bass_kernel_spmd(
        nc, in_maps, core_ids=list(range(N_CORES)), trace=True, **trace_kwargs
    )
    out = np.concatenate([r["y"] for r in res.results], axis=0)
    return out, res


# revision 21
# speedup vs baseline: 1.0053x; 1.0053x over previous
"""Trainium2 Bass kernel for y = inputs @ weights.T + bias.

Shapes: inputs [8192, 4096] f32, weights [4096, 4096] f32, bias [4096] f32,
output [8192, 4096] f32.

Strategy:
- Data-parallel across 8 NeuronCores: each core computes 1024 rows of the
  output; weights/bias are replicated.
- Host pre-transposes inputs and weights to K-major layout and converts them
  to bf16 (rel err ~2e-3 at K=4096, tolerance is 2e-2). bf16 halves DMA
  traffic/SBUF footprint vs fp32r and enables Fast Weight Load on the PE
  (LDWEIGHTS ~2x faster), while matmul throughput is the same 1 col/cycle.
- Per core: cache the x-slice KxM [4096,1024] bf16 in SBUF (8.4 MB), stream
  W in [128,512] bf16 tiles, 8 PSUM banks accumulate fp32 over K, bias added
  on DVE during PSUM eviction, y written back as fp32.
- DMA queues are dedicated per stream (w: sync+scalar, x: gpsimd+vector,
  y: gpsimd) so the w-tile stream never suffers head-of-line blocking
  behind x/y transfers (which caused a periodic one-matmul stall).
- A short burst of dummy matmuls on a memset tile right after the engine
  preamble warms the PE HAM clock gate (K=4/8 -> 8/8) while the first real
  tiles are still in flight, so real matmuls run at 2.4 GHz from the start.
- The last n-block runs mb-major on a prefetched w-set so 7 of its 8 PSUM
  evictions/output DMAs overlap the matmul stream; only the final [128,512]
  tile drains after the last matmul (split in halves to pipeline with DMA).
"""

import numpy as np
import ml_dtypes

import concourse.bacc as bacc
import concourse.mybir as mybir
import concourse.tile as tile
from concourse.bass_utils import run_bass_kernel_spmd

N_CORES = 8
N_FULL = 8192  # input rows
K_DIM = 4096  # contraction (in features)
O_DIM = 4096  # out features
M = N_FULL // N_CORES  # rows per core (1024)
P = 128
KO = K_DIM // P  # 32 k-tiles
N_TILE = 512  # moving free dim per matmul (1 PSUM bank of fp32)
N_BLOCKS = O_DIM // N_TILE  # 8
M_BLOCKS = M // P  # 8
N_WARM = 24  # dummy warm-up matmuls (~4.5us; extra margin for x-load jitter)

_nc_cache = None


def _build():
    nc = bacc.Bacc(target_bir_lowering=False)

    xT = nc.dram_tensor("xT", [K_DIM, M], mybir.dt.bfloat16, kind="ExternalInput")
    wT = nc.dram_tensor("wT", [K_DIM, O_DIM], mybir.dt.bfloat16, kind="ExternalInput")
    biasr = nc.dram_tensor("biasr", [P, O_DIM], mybir.dt.float32, kind="ExternalInput")
    y = nc.dram_tensor("y", [M, O_DIM], mybir.dt.float32, kind="ExternalOutput")

    xT3 = xT.ap().rearrange("(ko p) m -> p ko m", p=P)
    wT3 = wT.ap().rearrange("(ko p) n -> p ko n", p=P)
    y3 = y.ap().rearrange("(mb p) n -> p mb n", p=P)

    with tile.TileContext(nc) as tc:
        with (
            tc.tile_pool(name="persist", bufs=1) as persist,
            tc.tile_pool(name="wpool", bufs=6) as wpool,
            tc.tile_pool(name="opool", bufs=8) as opool,
            tc.tile_pool(name="ohpool", bufs=4) as ohpool,
            tc.tile_pool(name="psum", bufs=1, space="PSUM") as psum_pool,
        ):
            psums = [
                psum_pool.tile(
                    [P, N_TILE], mybir.dt.float32, tag=f"ps{m}", name=f"ps{m}"
                )
                for m in range(M_BLOCKS)
            ]

            # PE warm-up: dummy matmuls on a memset tile, no DMA deps. They
            # run during the DMA latency of the first real tiles and flip the
            # HAM clock gate to full speed before real matmuls start.
            warm = persist.tile([P, 384], mybir.dt.bfloat16, tag="warm")
            nc.gpsimd.memset(warm[:], 0)
            for _ in range(N_WARM):
                nc.tensor.matmul(
                    psums[M_BLOCKS - 1][:, :256],
                    warm[:, :P],
                    warm[:, P:],
                    start=True,
                    stop=True,
                )

            # x cached in SBUF, one tile per k-slab so matmuls can start as
            # soon as their slab has landed. Only gpsimd/sync/scalar can
            # issue DMAs; sync+scalar are dedicated to the w stream so it
            # never queues behind x/y transfers. x rides gpsimd alone,
            # except slab 0's odd chunks which go out on scalar before any
            # w traffic exists there, halving the first matmul's wait.
            # The first four slabs arrive in fine-grained chunks split over
            # gpsimd+scalar (scalar's first w tile was moved to sync, so its
            # early window is free) - a late early slab would idle the PE
            # past the HAM window and trigger an expensive re-throttle.
            x_sb = []
            bias_sb = [None] * N_BLOCKS
            x_chunks = {0: 4, 1: 2, 2: 2, 3: 2, 4: 2, 5: 2}  # ko -> chunks
            for ko in range(KO):
                x_t = persist.tile([P, M], mybir.dt.bfloat16, tag=f"x{ko}")
                nchunk = x_chunks.get(ko, 1)
                csz = M // nchunk
                for c in range(nchunk):
                    xeng = nc.scalar if c % 2 == 1 else nc.gpsimd
                    xeng.dma_start(
                        x_t[:, c * csz : (c + 1) * csz],
                        xT3[:, ko, c * csz : (c + 1) * csz],
                    )
                x_sb.append(x_t)

            # bias, then the whole w-set of the last n-block, prefetched on
            # gpsimd (idle once x is cached) so nb=7 can run mb-major with
            # zero DMA dependencies.
            for nb in range(N_BLOCKS):
                b_t = persist.tile([P, N_TILE], mybir.dt.float32, tag=f"bias{nb}")
                nc.gpsimd.dma_start(
                    b_t[:], biasr.ap()[:, nb * N_TILE : (nb + 1) * N_TILE]
                )
                bias_sb[nb] = b_t

            LAST = N_BLOCKS - 1
            w_last = []

            # n-blocks 0..6: ko-major, streaming w tiles on sync/scalar.
            for nb in range(N_BLOCKS - 1):
                if nb == 1:
                    # prefetch the last n-block's w set now: the x load is
                    # done and gpsimd is idle, whereas during nb=0 the
                    # per-core HBM demand is already at its cap.
                    for ko in range(KO):
                        wl_t = persist.tile(
                            [P, N_TILE], mybir.dt.bfloat16, tag=f"wl{ko}"
                        )
                        nc.gpsimd.dma_start(
                            wl_t[:], wT3[:, ko, LAST * N_TILE : (LAST + 1) * N_TILE]
                        )
                        w_last.append(wl_t)
                for ko in range(KO):
                    w_t = wpool.tile([P, N_TILE], mybir.dt.bfloat16, tag="w")
                    # ko parity splits w over sync/scalar; w01/w03 go to sync
                    # too so scalar's early x chunks don't delay them.
                    weng = nc.sync if (ko % 2 == 0 or (nb == 0 and ko in (1, 3))) else nc.scalar
                    if nb == 0 and ko == 0:
                        # halve the very first tile's DMA latency
                        h = N_TILE // 2
                        weng.dma_start(w_t[:, :h], wT3[:, ko, :h])
                        weng.dma_start(w_t[:, h:], wT3[:, ko, h : N_TILE])
                    else:
                        weng.dma_start(
                            w_t[:], wT3[:, ko, nb * N_TILE : (nb + 1) * N_TILE]
                        )
                    for mb in range(M_BLOCKS):
                        nc.tensor.matmul(
                            psums[mb][:],
                            x_sb[ko][:, mb * P : (mb + 1) * P],
                            w_t[:],
                            start=(ko == 0),
                            stop=(ko == KO - 1),
                        )
                for mb in range(M_BLOCKS):
                    o_t = opool.tile([P, N_TILE], mybir.dt.float32, tag="o")
                    nc.vector.tensor_add(o_t[:], psums[mb][:], bias_sb[nb][:])
                    nc.gpsimd.dma_start(
                        y3[:, mb, nb * N_TILE : (nb + 1) * N_TILE], o_t[:]
                    )

            # Last n-block: mb-major over the prefetched w-set. Each psum
            # bank finishes 32 matmuls before the next starts, so its
            # eviction + y DMA overlap the remaining matmuls; only mb=7
            # drains after the final matmul, in halves to pipeline with DMA.
            H = N_TILE // 2
            for mb in range(M_BLOCKS):
                for ko in range(KO):
                    if mb == M_BLOCKS - 1 and ko == KO - 1:
                        # split the final matmul in halves so the first
                        # half's eviction overlaps the second half
                        for h in range(2):
                            nc.tensor.matmul(
                                psums[mb][:, h * H : (h + 1) * H],
                                x_sb[ko][:, mb * P : (mb + 1) * P],
                                w_last[ko][:, h * H : (h + 1) * H],
                                start=False,
                                stop=True,
                            )
                        continue
                    nc.tensor.matmul(
                        psums[mb][:],
                        x_sb[ko][:, mb * P : (mb + 1) * P],
                        w_last[ko][:],
                        start=(ko == 0),
                        stop=(ko == KO - 1),
                    )
                for h in range(2):
                    o_t = ohpool.tile([P, H], mybir.dt.float32, tag="oh")
                    nc.vector.tensor_add(
                        o_t[:],
                        psums[mb][:, h * H : (h + 1) * H],
                        bias_sb[LAST][:, h * H : (h + 1) * H],
                    )
                    oeng = nc.sync if h % 2 == 0 else nc.scalar
                    oeng.dma_start(
                        y3[
                            :,
                            mb,
                            LAST * N_TILE + h * H : LAST * N_TILE + (h + 1) * H,
                        ],
                        o_t[:],
                    )

    nc.compile()
    return nc


def _get_nc():
    global _nc_cache
    if _nc_cache is None:
        _nc_cache = _build()
    return _nc_cache


def _make_in_maps(inputs, weights, bias):
    x = np.asarray(inputs, dtype=np.float32)
    w = np.asarray(weights, dtype=np.float32)
    b = np.asarray(bias, dtype=np.float32)

    xbf = np.ascontiguousarray(x).astype(ml_dtypes.bfloat16)
    wbf = np.ascontiguousarray(w).astype(ml_dtypes.bfloat16)
    xT = xbf.T  # [K, N_FULL] view
    wT = np.ascontiguousarray(wbf.T)  # [K, O]
    br = np.ascontiguousarray(np.broadcast_to(b[None, :], (P, O_DIM)))

    in_maps = []
    for c in range(N_CORES):
        xTc = np.ascontiguousarray(xT[:, c * M : (c + 1) * M])
        in_maps.append({"xT": xTc, "wT": wT, "biasr": br})
    return in_maps


def kernel(**inputs):
    nc = _get_nc()
    in_maps = _make_in_maps(inputs["inputs"], inputs["weights"], inputs["bias"])
    res = run_bass_kernel_spmd(nc, in_maps, core_ids=list(range(N_CORES)))
    return np.concatenate([r["y"] for r in res.results], axis=0)


def run_traced(inputs, weights, bias, **trace_kwargs):
    """Used by test.py: same computation, returns (output, BassKernelResults)."""
    nc = _get_nc()
    in_maps = _make_in_maps(inputs, weights, bias)
    res = run_bass_kernel_spmd(
        nc, in_maps, core_ids=list(range(N_CORES)), trace=True, **trace_kwargs
    )
    out = np.concatenate([r["y"] for r in res.results], axis=0)
    return out, res


# revision 22
# speedup vs baseline: 1.1147x; 1.1087x over previous
"""Trainium2 Bass kernel for y = inputs @ weights.T + bias.

Shapes: inputs [8192, 4096] f32, weights [4096, 4096] f32, bias [4096] f32,
output [8192, 4096] f32.

Strategy:
- Data-parallel across 8 NeuronCores: each core computes 1024 rows of the
  output; weights/bias are replicated.
- Host pre-transposes inputs and weights to K-major layout and converts them
  to bf16 (rel err ~2e-3 at K=4096, tolerance is 2e-2). bf16 halves DMA
  traffic/SBUF footprint vs fp32r and enables Fast Weight Load on the PE
  (LDWEIGHTS ~2x faster), while matmul throughput is the same 1 col/cycle.
- Per core: cache the x-slice KxM [4096,1024] bf16 in SBUF (8.4 MB), stream
  W in [128,512] bf16 tiles, 8 PSUM banks accumulate fp32 over K, bias added
  on DVE during PSUM eviction, y written back as fp32.
- DMA queues are dedicated per stream (w: sync+scalar, x: gpsimd+vector,
  y: gpsimd) so the w-tile stream never suffers head-of-line blocking
  behind x/y transfers (which caused a periodic one-matmul stall).
- A short burst of dummy matmuls on a memset tile right after the engine
  preamble warms the PE HAM clock gate (K=4/8 -> 8/8) while the first real
  tiles are still in flight, so real matmuls run at 2.4 GHz from the start.
- The last n-block runs mb-major on a prefetched w-set so 7 of its 8 PSUM
  evictions/output DMAs overlap the matmul stream; only the final [128,512]
  tile drains after the last matmul (split in halves to pipeline with DMA).
"""

import numpy as np
import ml_dtypes

import concourse.bacc as bacc
import concourse.mybir as mybir
import concourse.tile as tile
from concourse.bass_utils import run_bass_kernel_spmd

N_CORES = 8
N_FULL = 8192  # input rows
K_DIM = 4096  # contraction (in features)
O_DIM = 4096  # out features
M = N_FULL // N_CORES  # rows per core (1024)
P = 128
KO = K_DIM // P  # 32 k-tiles
N_TILE = 512  # moving free dim per matmul (1 PSUM bank of fp32)
N_BLOCKS = O_DIM // N_TILE  # 8
M_BLOCKS = M // P  # 8
N_WARM = 17  # dummy warm-up matmuls (~3.7us at the cold 1.2 GHz clock)

_nc_cache = None


def _build():
    nc = bacc.Bacc(target_bir_lowering=False)

    xT = nc.dram_tensor("xT", [K_DIM, M], mybir.dt.bfloat16, kind="ExternalInput")
    wT = nc.dram_tensor("wT", [K_DIM, O_DIM], mybir.dt.bfloat16, kind="ExternalInput")
    biasr = nc.dram_tensor("biasr", [P, O_DIM], mybir.dt.float32, kind="ExternalInput")
    y = nc.dram_tensor("y", [M, O_DIM], mybir.dt.float32, kind="ExternalOutput")

    xT3 = xT.ap().rearrange("(ko p) m -> p ko m", p=P)
    wT3 = wT.ap().rearrange("(ko p) n -> p ko n", p=P)
    y3 = y.ap().rearrange("(mb p) n -> p mb n", p=P)

    with tile.TileContext(nc) as tc:
        with (
            tc.tile_pool(name="persist", bufs=1) as persist,
            tc.tile_pool(name="wpool", bufs=10) as wpool,
            tc.tile_pool(name="opool", bufs=8) as opool,
            tc.tile_pool(name="ohpool", bufs=4) as ohpool,
            tc.tile_pool(name="psum", bufs=1, space="PSUM") as psum_pool,
        ):
            psums = [
                psum_pool.tile(
                    [P, N_TILE], mybir.dt.float32, tag=f"ps{m}", name=f"ps{m}"
                )
                for m in range(M_BLOCKS)
            ]

            # PE warm-up: dummy matmuls on a memset tile, no DMA deps. They
            # run during the DMA latency of the first real tiles and flip the
            # HAM clock gate to full speed before real matmuls start.
            warm = persist.tile([P, 384], mybir.dt.bfloat16, tag="warm")
            nc.gpsimd.memset(warm[:], 0)
            for _ in range(N_WARM):
                nc.tensor.matmul(
                    psums[M_BLOCKS - 1][:, :256],
                    warm[:, :P],
                    warm[:, P:],
                    start=True,
                    stop=True,
                )

            # x cached in SBUF, one tile per k-slab so matmuls can start as
            # soon as their slab has landed. Only gpsimd/sync/scalar can
            # issue DMAs; sync+scalar are dedicated to the w stream so it
            # never queues behind x/y transfers. x rides gpsimd alone,
            # except slab 0's odd chunks which go out on scalar before any
            # w traffic exists there, halving the first matmul's wait.
            # The first four slabs arrive in fine-grained chunks split over
            # gpsimd+scalar (scalar's first w tile was moved to sync, so its
            # early window is free) - a late early slab would idle the PE
            # past the HAM window and trigger an expensive re-throttle.
            x_sb = []
            bias_sb = [None] * N_BLOCKS
            x_chunks = {0: 4, 1: 2, 2: 2, 3: 2}  # ko -> number of load chunks
            for ko in range(KO):
                x_t = persist.tile([P, M], mybir.dt.bfloat16, tag=f"x{ko}")
                nchunk = x_chunks.get(ko, 1)
                csz = M // nchunk
                for c in range(nchunk):
                    xeng = nc.scalar if c % 2 == 1 else nc.gpsimd
                    xeng.dma_start(
                        x_t[:, c * csz : (c + 1) * csz],
                        xT3[:, ko, c * csz : (c + 1) * csz],
                    )
                x_sb.append(x_t)

            # bias, then the whole w-set of the last n-block, prefetched on
            # gpsimd (idle once x is cached) so nb=7 can run mb-major with
            # zero DMA dependencies.
            for nb in range(N_BLOCKS):
                b_t = persist.tile([P, N_TILE], mybir.dt.float32, tag=f"bias{nb}")
                nc.gpsimd.dma_start(
                    b_t[:], biasr.ap()[:, nb * N_TILE : (nb + 1) * N_TILE]
                )
                bias_sb[nb] = b_t

            LAST = N_BLOCKS - 1
            w_last = []
            for ko in range(KO):
                wl_t = persist.tile([P, N_TILE], mybir.dt.bfloat16, tag=f"wl{ko}")
                nc.gpsimd.dma_start(
                    wl_t[:], wT3[:, ko, LAST * N_TILE : (LAST + 1) * N_TILE]
                )
                w_last.append(wl_t)

            # n-blocks 0..6: ko-major, streaming w tiles on sync/scalar.
            for nb in range(N_BLOCKS - 1):
                for ko in range(KO):
                    w_t = wpool.tile([P, N_TILE], mybir.dt.bfloat16, tag="w")
                    # ko parity splits w over sync/scalar; w01 goes to sync
                    # too so scalar's early x chunks don't delay it.
                    weng = nc.sync if (ko % 2 == 0 or (nb == 0 and ko == 1)) else nc.scalar
                    if nb == 0 and ko == 0:
                        # halve the very first tile's DMA latency
                        h = N_TILE // 2
                        weng.dma_start(w_t[:, :h], wT3[:, ko, :h])
                        weng.dma_start(w_t[:, h:], wT3[:, ko, h : N_TILE])
                    else:
                        weng.dma_start(
                            w_t[:], wT3[:, ko, nb * N_TILE : (nb + 1) * N_TILE]
                        )
                    for mb in range(M_BLOCKS):
                        nc.tensor.matmul(
                            psums[mb][:],
                            x_sb[ko][:, mb * P : (mb + 1) * P],
                            w_t[:],
                            start=(ko == 0),
                            stop=(ko == KO - 1),
                        )
                for mb in range(M_BLOCKS):
                    o_t = opool.tile([P, N_TILE], mybir.dt.float32, tag="o")
                    nc.vector.tensor_add(o_t[:], psums[mb][:], bias_sb[nb][:])
                    nc.gpsimd.dma_start(
                        y3[:, mb, nb * N_TILE : (nb + 1) * N_TILE], o_t[:]
                    )

            # Last n-block: mb-major over the prefetched w-set. Each psum
            # bank finishes 32 matmuls before the next starts, so its
            # eviction + y DMA overlap the remaining matmuls; only mb=7
            # drains after the final matmul, in halves to pipeline with DMA.
            H = N_TILE // 2
            for mb in range(M_BLOCKS):
                for ko in range(KO):
                    nc.tensor.matmul(
                        psums[mb][:],
                        x_sb[ko][:, mb * P : (mb + 1) * P],
                        w_last[ko][:],
                        start=(ko == 0),
                        stop=(ko == KO - 1),
                    )
                for h in range(2):
                    o_t = ohpool.tile([P, H], mybir.dt.float32, tag="oh")
                    nc.vector.tensor_add(
                        o_t[:],
                        psums[mb][:, h * H : (h + 1) * H],
                        bias_sb[LAST][:, h * H : (h + 1) * H],
                    )
                    oeng = nc.sync if h % 2 == 0 else nc.scalar
                    oeng.dma_start(
                        y3[
                            :,
                            mb,
                            LAST * N_TILE + h * H : LAST * N_TILE + (h + 1) * H,
                        ],
                        o_t[:],
                    )

    nc.compile()
    return nc


def _get_nc():
    global _nc_cache
    if _nc_cache is None:
        _nc_cache = _build()
    return _nc_cache


def _make_in_maps(inputs, weights, bias):
    x = np.asarray(inputs, dtype=np.float32)
    w = np.asarray(weights, dtype=np.float32)
    b = np.asarray(bias, dtype=np.float32)

    xbf = np.ascontiguousarray(x).astype(ml_dtypes.bfloat16)
    wbf = np.ascontiguousarray(w).astype(ml_dtypes.bfloat16)
    xT = xbf.T  # [K, N_FULL] view
    wT = np.ascontiguousarray(wbf.T)  # [K, O]
    br = np.ascontiguousarray(np.broadcast_to(b[None, :], (P, O_DIM)))

    in_maps = []
    for c in range(N_CORES):
        xTc = np.ascontiguousarray(xT[:, c * M : (c + 1) * M])
        in_maps.append({"xT": xTc, "wT": wT, "biasr": br})
    return in_maps


def kernel(**inputs):
    nc = _get_nc()
    in_maps = _make_in_maps(inputs["inputs"], inputs["weights"], inputs["bias"])
    res = run_bass_kernel_spmd(nc, in_maps, core_ids=list(range(N_CORES)))
    return np.concatenate([r["y"] for r in res.results], axis=0)


def run_traced(inputs, weights, bias, **trace_kwargs):
    """Used by test.py: same computation, returns (output, BassKernelResults)."""
    nc = _get_nc()
    in_maps = _make_in_maps(inputs, weights, bias)
    res = run_bass_kernel_spmd(
        nc, in_maps, core_ids=list(range(N_CORES)), trace=True, **trace_kwargs
    )
    out = np.concatenate([r["y"] for r in res.results], axis=0)
    return out, res


# revision 23
# speedup vs baseline: 1.1573x; 1.0382x over previous
"""Trainium2 Bass kernel for y = inputs @ weights.T + bias.

Shapes: inputs [8192, 4096] f32, weights [4096, 4096] f32, bias [4096] f32,
output [8192, 4096] f32.

Strategy:
- Data-parallel across 8 NeuronCores: each core computes 1024 rows of the
  output; weights/bias are replicated.
- Host pre-transposes inputs and weights to K-major layout and converts them
  to bf16 (rel err ~2e-3 at K=4096, tolerance is 2e-2). bf16 halves DMA
  traffic/SBUF footprint vs fp32r and enables Fast Weight Load on the PE
  (LDWEIGHTS ~2x faster), while matmul throughput is the same 1 col/cycle.
- Per core: cache the x-slice KxM [4096,1024] bf16 in SBUF (8.4 MB), stream
  W in [128,512] bf16 tiles, 8 PSUM banks accumulate fp32 over K, bias added
  on DVE during PSUM eviction, y written back as fp32.
- DMA queues are dedicated per stream (w: sync+scalar, x: gpsimd+vector,
  y: gpsimd) so the w-tile stream never suffers head-of-line blocking
  behind x/y transfers (which caused a periodic one-matmul stall).
- A short burst of dummy matmuls on a memset tile right after the engine
  preamble warms the PE HAM clock gate (K=4/8 -> 8/8) while the first real
  tiles are still in flight, so real matmuls run at 2.4 GHz from the start.
- The last n-block runs mb-major on a prefetched w-set so 7 of its 8 PSUM
  evictions/output DMAs overlap the matmul stream; only the final [128,512]
  tile drains after the last matmul (split in halves to pipeline with DMA).
"""

import numpy as np
import ml_dtypes

import concourse.bacc as bacc
import concourse.mybir as mybir
import concourse.tile as tile
from concourse.bass_utils import run_bass_kernel_spmd

N_CORES = 8
N_FULL = 8192  # input rows
K_DIM = 4096  # contraction (in features)
O_DIM = 4096  # out features
M = N_FULL // N_CORES  # rows per core (1024)
P = 128
KO = K_DIM // P  # 32 k-tiles
N_TILE = 512  # moving free dim per matmul (1 PSUM bank of fp32)
N_BLOCKS = O_DIM // N_TILE  # 8
M_BLOCKS = M // P  # 8
N_WARM = 17  # dummy warm-up matmuls (~3.7us at the cold 1.2 GHz clock)
KO_BF = 24  # k-slabs done in bf16 (ko 0..23)
T8 = (KO - KO_BF) // 2  # fp8 DoubleRow pairs covering k rows 3072..4095 (4)
K_BF = KO_BF * P  # 3072

_nc_cache = None


def _build():
    nc = bacc.Bacc(target_bir_lowering=False)

    xT = nc.dram_tensor("xT", [K_BF, M], mybir.dt.bfloat16, kind="ExternalInput")
    wT = nc.dram_tensor("wT", [K_BF, O_DIM], mybir.dt.bfloat16, kind="ExternalInput")
    x8 = nc.dram_tensor("x8", [T8 * P, 2 * M], mybir.dt.float8e4, kind="ExternalInput")
    w8 = nc.dram_tensor("w8", [T8 * P, 2 * O_DIM], mybir.dt.float8e4, kind="ExternalInput")
    biasr = nc.dram_tensor("biasr", [P, O_DIM], mybir.dt.float32, kind="ExternalInput")
    y = nc.dram_tensor("y", [M, O_DIM], mybir.dt.float32, kind="ExternalOutput")

    xT3 = xT.ap().rearrange("(ko p) m -> p ko m", p=P)
    wT3 = wT.ap().rearrange("(ko p) n -> p ko n", p=P)
    x84 = x8.ap().rearrange("(t p) (j m) -> p t j m", p=P, j=2)
    w85 = w8.ap().rearrange("(t p) (nb j n) -> p t nb j n", p=P, nb=N_BLOCKS, j=2)
    y3 = y.ap().rearrange("(mb p) n -> p mb n", p=P)

    with tile.TileContext(nc) as tc:
        with (
            tc.tile_pool(name="persist", bufs=1) as persist,
            tc.tile_pool(name="wpool", bufs=10) as wpool,
            tc.tile_pool(name="opool", bufs=8) as opool,
            tc.tile_pool(name="ohpool", bufs=4) as ohpool,
            tc.tile_pool(name="psum", bufs=1, space="PSUM") as psum_pool,
        ):
            psums = [
                psum_pool.tile(
                    [P, N_TILE], mybir.dt.float32, tag=f"ps{m}", name=f"ps{m}"
                )
                for m in range(M_BLOCKS)
            ]

            # PE warm-up: dummy matmuls on a memset tile, no DMA deps. They
            # run during the DMA latency of the first real tiles and flip the
            # HAM clock gate to full speed before real matmuls start.
            warm = persist.tile([P, 384], mybir.dt.bfloat16, tag="warm")
            nc.gpsimd.memset(warm[:], 0)
            for _ in range(N_WARM):
                nc.tensor.matmul(
                    psums[M_BLOCKS - 1][:, :256],
                    warm[:, :P],
                    warm[:, P:],
                    start=True,
                    stop=True,
                )

            # x cached in SBUF, one tile per k-slab so matmuls can start as
            # soon as their slab has landed. Only gpsimd/sync/scalar can
            # issue DMAs; sync+scalar are dedicated to the w stream so it
            # never queues behind x/y transfers. x rides gpsimd alone,
            # except slab 0's odd chunks which go out on scalar before any
            # w traffic exists there, halving the first matmul's wait.
            # The first four slabs arrive in fine-grained chunks split over
            # gpsimd+scalar (scalar's first w tile was moved to sync, so its
            # early window is free) - a late early slab would idle the PE
            # past the HAM window and trigger an expensive re-throttle.
            x_sb = []
            bias_sb = [None] * N_BLOCKS
            x_chunks = {0: 4, 1: 2, 2: 2, 3: 2}  # ko -> number of load chunks
            for ko in range(KO_BF):
                x_t = persist.tile([P, M], mybir.dt.bfloat16, tag=f"x{ko}")
                nchunk = x_chunks.get(ko, 1)
                csz = M // nchunk
                for c in range(nchunk):
                    xeng = nc.scalar if c % 2 == 1 else nc.gpsimd
                    xeng.dma_start(
                        x_t[:, c * csz : (c + 1) * csz],
                        xT3[:, ko, c * csz : (c + 1) * csz],
                    )
                x_sb.append(x_t)

            # fp8-packed x pairs (two k-slabs interleaved along the free dim)
            x8_sb = []
            for t in range(T8):
                x8_t = persist.tile([P, 2, M], mybir.dt.float8e4, tag=f"x8_{t}")
                nc.gpsimd.dma_start(x8_t[:], x84[:, t])
                x8_sb.append(x8_t)

            # bias, then the whole w-set of the last n-block, prefetched on
            # gpsimd (idle once x is cached) so nb=7 can run mb-major with
            # zero DMA dependencies.
            for nb in range(N_BLOCKS):
                b_t = persist.tile([P, N_TILE], mybir.dt.float32, tag=f"bias{nb}")
                nc.gpsimd.dma_start(
                    b_t[:], biasr.ap()[:, nb * N_TILE : (nb + 1) * N_TILE]
                )
                bias_sb[nb] = b_t

            LAST = N_BLOCKS - 1
            w_last = []
            for ko in range(KO_BF):
                wl_t = persist.tile([P, N_TILE], mybir.dt.bfloat16, tag=f"wl{ko}")
                nc.gpsimd.dma_start(
                    wl_t[:], wT3[:, ko, LAST * N_TILE : (LAST + 1) * N_TILE]
                )
                w_last.append(wl_t)
            w8_last = []
            for t in range(T8):
                wl8_t = persist.tile([P, 2, N_TILE], mybir.dt.float8e4, tag=f"wl8_{t}")
                nc.gpsimd.dma_start(wl8_t[:], w85[:, t, LAST])
                w8_last.append(wl8_t)

            # n-blocks 0..6: ko-major, streaming w tiles on sync/scalar.
            for nb in range(N_BLOCKS - 1):
                for ko in range(KO_BF):
                    w_t = wpool.tile([P, N_TILE], mybir.dt.bfloat16, tag="w")
                    # ko parity splits w over sync/scalar; w01 goes to sync
                    # too so scalar's early x chunks don't delay it.
                    weng = nc.sync if (ko % 2 == 0 or (nb == 0 and ko == 1)) else nc.scalar
                    if nb == 0 and ko == 0:
                        # halve the very first tile's DMA latency
                        h = N_TILE // 2
                        weng.dma_start(w_t[:, :h], wT3[:, ko, :h])
                        weng.dma_start(w_t[:, h:], wT3[:, ko, h : N_TILE])
                    else:
                        weng.dma_start(
                            w_t[:], wT3[:, ko, nb * N_TILE : (nb + 1) * N_TILE]
                        )
                    for mb in range(M_BLOCKS):
                        nc.tensor.matmul(
                            psums[mb][:],
                            x_sb[ko][:, mb * P : (mb + 1) * P],
                            w_t[:],
                            start=(ko == 0),
                            stop=False,
                        )
                for t in range(T8):
                    w8_t = wpool.tile([P, 2, N_TILE], mybir.dt.float8e4, tag="w8")
                    weng = nc.sync if t % 2 == 0 else nc.scalar
                    weng.dma_start(w8_t[:], w85[:, t, nb])
                    for mb in range(M_BLOCKS):
                        nc.tensor.matmul(
                            psums[mb][:],
                            x8_sb[t][:, :, mb * P : (mb + 1) * P],
                            w8_t[:],
                            start=False,
                            stop=(t == T8 - 1),
                            perf_mode=mybir.MatmulPerfMode.DoubleRow,
                        )
                for mb in range(M_BLOCKS):
                    o_t = opool.tile([P, N_TILE], mybir.dt.float32, tag="o")
                    nc.vector.tensor_add(o_t[:], psums[mb][:], bias_sb[nb][:])
                    nc.gpsimd.dma_start(
                        y3[:, mb, nb * N_TILE : (nb + 1) * N_TILE], o_t[:]
                    )

            # Last n-block: mb-major over the prefetched w-set. Each psum
            # bank finishes 32 matmuls before the next starts, so its
            # eviction + y DMA overlap the remaining matmuls; only mb=7
            # drains after the final matmul, in halves to pipeline with DMA.
            H = N_TILE // 2
            for mb in range(M_BLOCKS):
                for ko in range(KO_BF):
                    nc.tensor.matmul(
                        psums[mb][:],
                        x_sb[ko][:, mb * P : (mb + 1) * P],
                        w_last[ko][:],
                        start=(ko == 0),
                        stop=False,
                    )
                for t in range(T8):
                    nc.tensor.matmul(
                        psums[mb][:],
                        x8_sb[t][:, :, mb * P : (mb + 1) * P],
                        w8_last[t][:],
                        start=False,
                        stop=(t == T8 - 1),
                        perf_mode=mybir.MatmulPerfMode.DoubleRow,
                    )
                for h in range(2):
                    o_t = ohpool.tile([P, H], mybir.dt.float32, tag="oh")
                    nc.vector.tensor_add(
                        o_t[:],
                        psums[mb][:, h * H : (h + 1) * H],
                        bias_sb[LAST][:, h * H : (h + 1) * H],
                    )
                    oeng = nc.sync if h % 2 == 0 else nc.scalar
                    oeng.dma_start(
                        y3[
                            :,
                            mb,
                            LAST * N_TILE + h * H : LAST * N_TILE + (h + 1) * H,
                        ],
                        o_t[:],
                    )

    nc.compile()
    return nc


def _get_nc():
    global _nc_cache
    if _nc_cache is None:
        _nc_cache = _build()
    return _nc_cache


def _make_in_maps(inputs, weights, bias):
    import concourse.mybir as _mybir

    f8 = _mybir.dt.np(_mybir.dt.float8e4)
    x = np.asarray(inputs, dtype=np.float32)
    w = np.asarray(weights, dtype=np.float32)
    b = np.asarray(bias, dtype=np.float32)

    xbf = np.ascontiguousarray(x[:, :K_BF]).astype(ml_dtypes.bfloat16)
    wbf = np.ascontiguousarray(w[:, :K_BF]).astype(ml_dtypes.bfloat16)
    xT = xbf.T  # [K_BF, N_FULL] view
    wT = np.ascontiguousarray(wbf.T)  # [K_BF, O]
    br = np.ascontiguousarray(np.broadcast_to(b[None, :], (P, O_DIM)))

    # fp8 tail of K, packed for DoubleRow: two k-slabs per tile, j-major in
    # the free dim. Any (p, j) -> k mapping works as long as x and w match.
    xT8 = np.ascontiguousarray(x[:, K_BF:].T).astype(f8)  # [T8*2*P, N_FULL]
    wT8 = np.ascontiguousarray(w[:, K_BF:].T).astype(f8)  # [T8*2*P, O]
    # [t, j, p, n] -> [t, p, j, n] -> [(t p), (j n)]
    w8 = np.ascontiguousarray(
        wT8.reshape(T8, 2, P, N_BLOCKS, N_TILE)
        .transpose(0, 2, 3, 1, 4)
        .reshape(T8 * P, 2 * O_DIM)
    )

    in_maps = []
    for c in range(N_CORES):
        xTc = np.ascontiguousarray(xT[:, c * M : (c + 1) * M])
        x8c = np.ascontiguousarray(
            xT8[:, c * M : (c + 1) * M]
            .reshape(T8, 2, P, M)
            .transpose(0, 2, 1, 3)
            .reshape(T8 * P, 2 * M)
        )
        in_maps.append({"xT": xTc, "wT": wT, "biasr": br, "x8": x8c, "w8": w8})
    return in_maps


def kernel(**inputs):
    nc = _get_nc()
    in_maps = _make_in_maps(inputs["inputs"], inputs["weights"], inputs["bias"])
    res = run_bass_kernel_spmd(nc, in_maps, core_ids=list(range(N_CORES)))
    return np.concatenate([r["y"] for r in res.results], axis=0)


def run_traced(inputs, weights, bias, **trace_kwargs):
    """Used by test.py: same computation, returns (output, BassKernelResults)."""
    nc = _get_nc()
    in_maps = _make_in_maps(inputs, weights, bias)
    res = run_bass_kernel_spmd(
        nc, in_maps, core_ids=list(range(N_CORES)), trace=True, **trace_kwargs
    )
    out = np.concatenate([r["y"] for r in res.results], axis=0)
    return out, res
